# revision 21
# baseline (speedup 1.0000x reference)
"""Trainium2 Bass kernel for nn_CAModel (neural cellular automaton step).

Strategy: pure data-parallel over batch (16 samples -> 8 cores x 2).
v3: baseline's contiguous staging (sobel partials materialized shifted so
the S gathers are big-chunk 2D DMAs; x slab streamed from DRAM xcm), plus:
  - x/out in bf16 (halves HBM + update cost; ~0.3% rms, budget is 2e-2)
  - relu split ScalarE(3/4) / VectorE(1/4), b1 folded into the matmul via
    a const-1 row when b1 != 0 (zero in the graded model)
  - x-update on GpSimd, evac at psum-bank granularity
  - per-strip staging, tails emitted last (no FIFO head-of-line blocks)
Host does layout transforms only (free); HW exec time is what counts.
"""

import numpy as np

# ---------------------------------------------------------------- constants
B, C, H, W = 16, 16, 256, 256
NCORES = 8
SPC = B // NCORES          # samples per core
HWPX = H * W               # 65536 pixels per sample
PITCH = 258                # padded row pitch (wrap col + 256 + wrap col)
NROWH = 34                 # rows -1..32 (halo top/bottom) for x_bf
XBF_F = NROWH * PITCH      # 8772
SOB_F = 32 * PITCH         # 8256 (rows 0..31 padded)
PIX_F = 8192               # 32*256 pixels per strip
NT = HWPX // 128           # 512 pixel-tiles per sample
NSTRIP = 8                 # strips of 32 rows
SUB = 1024                 # psum subchunk (one psh tile)
KROWS = 48                 # mm1 contraction: x, V=u(w+1)-u(w-1), E (+1 if b1)
RELU_PAT = (0, 0, 0, 0, 1)  # per-subchunk relu engine: 0=ScalarE, 1=VectorE
ALPHA_TH = 0.1
FIRE = 0.5

_BUILT = None


# ------------------------------------------------------------- host layouts
def _bf16():
    import ml_dtypes
    return ml_dtypes.bfloat16


def _prep_xbf(x):
    """x: [B, C, H, W] f32 -> [B, 128, XBF_F] bf16 strip layout w/ halo+wrap.

    partition p = hb*16 + c ; free = (r, pc): r = hl+1 for hl in -1..32,
    pc: 0 <-> w=255, 1..256 <-> w=0..255, 257 <-> w=0.   h = hb*32 + hl mod 256
    """
    bf16 = _bf16()
    xb = x.astype(bf16)                                   # [B, C, H, W]
    hidx = (np.arange(-1, 33)[None, :] + 32 * np.arange(8)[:, None]) % 256
    xr = xb[:, :, hidx, :]                                # [B, C, 8, 34, W]
    out = np.empty((B, 8, C, NROWH, PITCH), dtype=bf16)
    out[:, :, :, :, 1:257] = np.transpose(xr, (0, 2, 1, 3, 4))
    out[:, :, :, :, 0] = np.transpose(xr[:, :, :, :, 255], (0, 2, 1, 3))
    out[:, :, :, :, 257] = np.transpose(xr[:, :, :, :, 0], (0, 2, 1, 3))
    return np.ascontiguousarray(out.reshape(B, 128, XBF_F))


def _prep_xcm(x):
    """x [B,C,H,W] f32 -> [B, 8, 16, 8192] bf16: per-strip channel-major."""
    bf16 = _bf16()
    xs = x.reshape(B, C, NSTRIP, PIX_F).transpose(0, 2, 1, 3)
    return np.ascontiguousarray(xs.astype(bf16))


def _prep_xt(x):
    """x: [B, C, H, W] f32 -> pixel-major [B, 128, 8192] bf16.

    xt[b, p, 16*t + c] = x[b, c, pix] with pix = 128*t + p (raster order).
    """
    bf16 = _bf16()
    xf = x.reshape(B, C, HWPX).transpose(0, 2, 1)         # [B, pix, C]
    xf = xf.reshape(B, NT, 128, C).transpose(0, 2, 1, 3)  # [B, p, t, c]
    return np.ascontiguousarray(xf.reshape(B, 128, NT * C).astype(bf16))


def _prep_randt(rv):
    """rand_vals [B, 1, H, W] -> [B, 128, NT] f32, rt[b, p, t] = rv[b, pix]."""
    rf = rv.reshape(B, HWPX).reshape(B, NT, 128).transpose(0, 2, 1)
    return np.ascontiguousarray(rf.astype(np.float32))


def _unprep_out(op):
    """out_pm [B, 128, 8192] bf16 -> [B, C, H, W] f32."""
    o = op.astype(np.float32).reshape(B, 128, NT, C).transpose(0, 2, 1, 3)
    o = o.reshape(B, HWPX, C).transpose(0, 2, 1)
    return np.ascontiguousarray(o.reshape(B, C, H, W))


def _prep_weights(w1, b1, w2, b2):
    bf16 = _bf16()
    w1 = np.asarray(w1, np.float32)
    w2 = np.asarray(w2, np.float32)
    b1 = np.asarray(b1, np.float32)
    # S rows: [x; V; E] with u = x(h-1)+2x+x(h+1), V = u(w+1)-u(w-1),
    # d = x(h+1)-x(h-1), E = d(w-1)+2d(w)+d(w+1).
    # pdx = 0.125*V ; pdy = 0.125*E
    wid, wdx, wdy = w1[0::3], w1[1::3], w1[2::3]
    parts = [wid, 0.125 * wdx, 0.125 * wdy]
    if np.any(b1 != 0.0):
        parts.append(b1.reshape(1, 128))
    w1e = np.concatenate(parts, axis=0)                   # [48 or 49, 128]
    # duplicate into rows 64.. for the second PE row-tile
    w1d = np.zeros((128, 128), np.float32)
    kr = w1e.shape[0]
    w1d[0:kr] = w1e
    w1d[64:64 + kr] = w1e
    return (np.ascontiguousarray(w1d.astype(bf16)),
            np.ascontiguousarray(w2.astype(bf16)),
            np.asarray(b2, np.float32).reshape(16))


# ------------------------------------------------------------- build module
def _build(b1_nonzero, b2_nonzero):
    import concourse.bass as bass
    import concourse.bacc as bacc
    import concourse.mybir as mybir
    import concourse.tile as tile

    dt = mybir.dt
    op = mybir.AluOpType
    AF = mybir.ActivationFunctionType

    nc = bacc.Bacc("TRN2", target_bir_lowering=False, debug=False)

    kr = KROWS + (1 if b1_nonzero else 0)
    xbf_d = nc.dram_tensor("xbf", (SPC, 128, XBF_F), dt.bfloat16, kind="ExternalInput")
    xcm_d = nc.dram_tensor("xcm", (SPC, NSTRIP, 16, PIX_F), dt.bfloat16, kind="ExternalInput")
    xt_d = nc.dram_tensor("xt", (SPC, 128, PIX_F), dt.bfloat16, kind="ExternalInput")
    rt_d = nc.dram_tensor("rt", (SPC, 128, NT), dt.float32, kind="ExternalInput")
    w1_d = nc.dram_tensor("w1e", (128, 128), dt.bfloat16, kind="ExternalInput")
    w2_d = nc.dram_tensor("w2e", (128, 16), dt.bfloat16, kind="ExternalInput")
    b2_d = nc.dram_tensor("b2e", (1, 16), dt.float32, kind="ExternalInput")
    out_d = nc.dram_tensor("outp", (SPC, 128, PIX_F), dt.bfloat16, kind="ExternalOutput")

    with tile.TileContext(nc) as tc:
        with (
            tc.tile_pool(name="wpool", bufs=1) as wpool,
            tc.tile_pool(name="xbf", bufs=1) as p_xbf,
            tc.tile_pool(name="pads", bufs=1) as p_pads,
            tc.tile_pool(name="sob", bufs=2) as p_sob,
            tc.tile_pool(name="stage", bufs=3) as p_stage,
            tc.tile_pool(name="hsb", bufs=3) as p_hsb,
            tc.tile_pool(name="xt", bufs=2) as p_xt,
            tc.tile_pool(name="dxm", bufs=1) as p_dxm,
            tc.tile_pool(name="small", bufs=2) as p_small,
            tc.tile_pool(name="small1", bufs=1) as p_small1,
            tc.tile_pool(name="pscr", bufs=1) as p_pscr,
            tc.tile_pool(name="psh", bufs=3, space=bass.MemorySpace.PSUM) as p_psh,
            tc.tile_pool(name="psdx", bufs=2, space=bass.MemorySpace.PSUM) as p_psdx,
        ):
            w1_sb = wpool.tile([128, 128], dt.bfloat16, tag="w1")
            nc.sync.dma_start(w1_sb[:], w1_d.ap())
            w2_sb = wpool.tile([128, 16], dt.bfloat16, tag="w2")
            nc.sync.dma_start(w2_sb[:], w2_d.ap())
            ones_sb = None
            if b1_nonzero:
                ones_sb = wpool.tile([1, PIX_F // 2], dt.bfloat16, tag="ones")
                nc.vector.memset(ones_sb[:], 1.0)
            # gpsimd ucode warmup (first TT call pays ucode load)
            gwarm = wpool.tile([128, 2], dt.bfloat16, tag="gwarm")
            nc.vector.memset(gwarm[:], 0.0)
            nc.gpsimd.tensor_tensor(gwarm[:, 0:1], gwarm[:, 0:1], gwarm[:, 1:2], op.mult)
            nc.gpsimd.tensor_tensor(gwarm[:, 0:1], gwarm[:, 0:1], gwarm[:, 1:2], op.add)
            b2_sb = None
            if b2_nonzero:
                b2_sb = wpool.tile([128, 16], dt.float32, tag="b2")
                nc.sync.dma_start(b2_sb[:], b2_d.ap().broadcast_to([128, 16]))

            def emit_head(s):
                """Loads + sobel partials: UP1/UM1 (shifted u) and E (pdy),
                materialized CONTIGUOUS so staging gathers are cheap DMAs."""
                st = {}
                rt = p_pscr.tile([128, NT], dt.float32, tag="rt")
                nc.sync.dma_start(rt[:], rt_d.ap()[s])
                xbf = p_xbf.tile([128, XBF_F], dt.bfloat16, tag="xbf")
                # load halo rows 0..18 first so sobel half 0 starts early
                nc.scalar.dma_start(xbf[:, 0:18 * PITCH],
                                    xbf_d.ap()[s][:, 0:18 * PITCH])
                nc.scalar.dma_start(xbf[:, 18 * PITCH:],
                                    xbf_d.ap()[s][:, 18 * PITCH:])
                xt = p_xt.tile([128, PIX_F], dt.bfloat16, tag="xt")
                nc.scalar.dma_start(xt[:], xt_d.ap()[s])
                xbf3 = xbf.rearrange("p (r q) -> p r q", q=PITCH)  # [128,34,258]

                V = p_sob.tile([128, PIX_F], dt.bfloat16, tag="V")
                E = p_sob.tile([128, PIX_F], dt.bfloat16, tag="E")
                V3 = V.rearrange("p (r w) -> p r w", w=W)
                E3 = E.rearrange("p (r w) -> p r w", w=W)
                Apad = p_pads.tile([128, SOB_F], dt.bfloat16, tag="A")
                A3 = Apad.rearrange("p (r q) -> p r q", q=PITCH)
                X2 = p_pads.tile([128, SOB_F], dt.bfloat16, tag="X2")
                X23 = X2.rearrange("p (r q) -> p r q", q=PITCH)
                Dpad = p_pads.tile([128, SOB_F], dt.bfloat16, tag="X2")
                D3 = Dpad.rearrange("p (r q) -> p r q", q=PITCH)
                D2 = p_pads.tile([128, SOB_F], dt.bfloat16, tag="A")
                D23 = D2.rearrange("p (r q) -> p r q", q=PITCH)
                for hh in range(2):
                    rr = slice(16 * hh, 16 * hh + 16)
                    x_up = xbf3[:, 16 * hh:16 * hh + 16, :]
                    x_mid = xbf3[:, 16 * hh + 1:16 * hh + 17, :]
                    x_dn = xbf3[:, 16 * hh + 2:16 * hh + 18, :]
                    # d path first (E is the last slab consumers wait on):
                    # D_pad = x_dn - x_up, E = D(w-1)+D(w+1), D2 = 2*D,
                    # E += D2(w)
                    nc.vector.tensor_tensor(D3[:, rr, :], x_dn, x_up, op.subtract)
                    nc.vector.tensor_tensor(E3[:, rr, :], D3[:, rr, 0:256],
                                            D3[:, rr, 2:258], op.add)
                    nc.vector.tensor_scalar(D23[:, rr, :], D3[:, rr, :], 2.0,
                                            None, op.mult)
                    nc.vector.tensor_tensor(E3[:, rr, :], D23[:, rr, 1:257],
                                            E3[:, rr, :], op.add)
                    # u path: U_pad = x_up + x_dn + 2*x_mid (in A's slot),
                    # then V = U_pad(w+1) - U_pad(w-1) contiguous
                    nc.vector.tensor_scalar(X23[:, rr, :], x_mid, 2.0, None, op.mult)
                    nc.vector.tensor_tensor(A3[:, rr, :], x_up, x_dn, op.add)
                    nc.vector.tensor_tensor(A3[:, rr, :], A3[:, rr, :],
                                            X23[:, rr, :], op.add)
                    nc.vector.tensor_tensor(V3[:, rr, :], A3[:, rr, 2:258],
                                            A3[:, rr, 0:256], op.subtract)

                xt3 = xt.rearrange("p (t c) -> p t c", c=C)
                st.update(xt=xt, xt3=xt3, V=V, E=E, rt=rt)
                return st

            def emit_mid(s, st, strips):
                """Per-strip staging (A-block rows 0..kr: even 512-px chunks,
                B-block rows 64..64+kr: odd chunks), row-tiled mm1 pairs,
                relu, mm2, psdx evac at bank (4096 px) granularity."""
                V, E = st["V"], st["E"]
                xt, xt3 = st["xt"], st["xt3"]
                if "alphaN" in st:
                    alphaN, DA = st["alphaN"], st["DA"]
                else:
                    alphaN = p_small.tile([128, NT], dt.bfloat16, tag="alN")
                    DA = p_small.tile([128, NT], dt.bfloat16, tag="DA")
                    st["alphaN"] = alphaN
                    st["DA"] = DA
                psdx = None
                for hb in strips:
                    pp = slice(16 * hb, 16 * hb + 16)
                    S = p_stage.tile([64 + kr, PIX_F // 2], dt.bfloat16, tag="S")
                    S3 = S.rearrange("p (a e) -> p a e", e=512)   # [112,8,512]
                    xc3 = xcm_d.ap()[s, hb].rearrange("c (a e) -> c a e", e=512)
                    V3s = V[pp, :].rearrange("c (a e) -> c a e", e=512)
                    E3s = E[pp, :].rearrange("c (a e) -> c a e", e=512)
                    # strip 0: stage per half so mm1 starts before the
                    # second sobel half is done
                    halves = ((0, 4), (4, 8)) if hb == 0 else ((0, 8),)
                    for a0, a1 in halves:
                        aa = slice(a0, a1)
                        e0, e1 = 2 * a0, 2 * a1
                        nc.sync.dma_start(S3[0:16, aa], xc3[:, e0:e1:2, :])
                        nc.sync.dma_start(S3[64:80, aa], xc3[:, e0 + 1:e1:2, :])
                        nc.gpsimd.dma_start(S3[16:32, aa], V3s[:, e0:e1:2, :])
                        nc.gpsimd.dma_start(S3[80:96, aa], V3s[:, e0 + 1:e1:2, :])
                        nc.scalar.dma_start(S3[32:48, aa], E3s[:, e0:e1:2, :])
                        nc.scalar.dma_start(S3[96:112, aa], E3s[:, e0 + 1:e1:2, :])
                    if b1_nonzero:
                        nc.sync.dma_start(S[48:49, :], ones_sb[:])
                        nc.sync.dma_start(S[112:113, :], ones_sb[:])

                    for j in range(PIX_F // SUB):
                        c0 = 512 * j
                        psh = p_psh.tile([128, SUB], dt.float32, tag="psh")
                        nc.tensor.matmul(psh[:, 0:512],
                                         w1_sb[0:kr, :], S[0:kr, c0:c0 + 512])
                        nc.tensor.matmul(psh[:, 512:1024],
                                         w1_sb[64:64 + kr, :],
                                         S[64:64 + kr, c0:c0 + 512])
                        hsb = p_hsb.tile([128, SUB], dt.bfloat16, tag="hsb")
                        g = hb * (PIX_F // SUB) + j    # subchunk 0..63
                        if hb < 2 or RELU_PAT[g % len(RELU_PAT)] == 0:
                            nc.scalar.activation(hsb[:], psh[:], AF.Relu)
                        else:
                            nc.vector.tensor_scalar(hsb[:], psh[:],
                                                    0.0, None, op.max)
                        if g % 4 == 0:
                            psdx = p_psdx.tile([128, 512], dt.float32, tag="psdx")
                        for t_loc in range(8):
                            tt = (g % 4) * 8 + t_loc
                            nc.tensor.matmul(
                                psdx[:, 16 * tt:16 * tt + 16],
                                hsb[:, 128 * t_loc:128 * (t_loc + 1)],
                                w2_sb[:])
                        if g % 4 == 3:
                            bk = g // 4    # psdx bank 0..15
                            DXM = p_dxm.tile([128, 512], dt.bfloat16, tag="DXM")
                            _evac_bank(nc, psdx, st["rt"], xt, xt3, DXM, alphaN,
                                       DA, bk, b2_sb, op, dt)

            def emit_tail(s, st):
                """Post-life pool, life mask, final multiply, store."""
                xt = st["xt"]
                alphaP = p_small1.tile([128, NT], dt.bfloat16, tag="alP")
                nc.vector.tensor_tensor(alphaP[:], st["alphaN"][:], st["DA"][:],
                                        op.subtract)
                preM = p_small1.tile([128, NT], dt.bfloat16, tag="preM")
                _pool_and_thresh(nc, p_pscr, alphaP, preM, op, dt)
                postM = p_small1.tile([128, NT], dt.bfloat16, tag="postM")
                _pool_and_thresh(nc, p_pscr, st["alphaN"], postM, op, dt)
                life = p_small1.tile([128, NT], dt.bfloat16, tag="life")
                eng = nc.vector if s == SPC - 1 else nc.gpsimd
                eng.tensor_tensor(life[:], preM[:], postM[:], op.mult)
                eng.tensor_tensor(
                    xt.rearrange("p (t c) -> p t c", c=C),
                    xt.rearrange("p (t c) -> p t c", c=C),
                    life[:].broadcast_to([128, NT, C]), op.mult)
                nc.gpsimd.dma_start(out_d.ap()[s], xt[:])

            def _evac_bank(nc, psdx, rt, xt, xt3, DXM, alphaN, DA, bk,
                           b2_sb, op, dt):
                """One filled psdx bank (4096 px = 32 tiles): update mask for
                this bank, masked dx -> DXM (bf16), alpha delta -> DA chunk,
                alphaN chunk, x += dx*um in place (bf16)."""
                ps3 = psdx.rearrange("p (t c) -> p t c", c=C)     # [128,32,16]
                sl32 = slice(32 * bk, 32 * bk + 32)
                umk = p_small1.tile([128, 32], dt.bfloat16, tag="umk")
                nc.vector.tensor_scalar(umk[:], rt[:, sl32], FIRE, None, op.is_lt)
                if b2_sb is not None:
                    nc.vector.tensor_tensor(
                        ps3[:], ps3[:],
                        b2_sb[:].rearrange("p c -> p 1 c").broadcast_to([128, 32, C]),
                        op.add)
                dxm3 = DXM.rearrange("p (t c) -> p t c", c=C)
                nc.vector.tensor_tensor(dxm3, ps3[:],
                                        umk[:].broadcast_to([128, 32, C]), op.mult)
                nc.vector.tensor_tensor(DA[:, sl32], ps3[:, :, 3], umk[:], op.mult)
                nc.vector.tensor_tensor(alphaN[:, sl32], DA[:, sl32],
                                        xt3[:, sl32, 3], op.add)
                sl = slice(512 * bk, 512 * (bk + 1))
                nc.gpsimd.tensor_tensor(xt[:, sl], xt[:, sl], DXM[:, :], op.add)

            def _pool_and_thresh(nc, pool, alpha, outM, op, dt):
                """3x3 circular max-pool on pixel-major alpha [128, NT] then
                > ALPHA_TH.  pix = 128*t + p: w-neighbors = partition +-1,
                h-neighbors = free -+2 (parity-interleaved wrap)."""
                bf = dt.bfloat16
                aL = pool.tile([128, NT], bf, tag="aL")
                aR = pool.tile([128, NT], bf, tag="aR")
                nc.sync.dma_start(aL[1:128, :], alpha[0:127, :])
                nc.sync.dma_start(aR[0:127, :], alpha[1:128, :])
                eL = pool.tile([1, NT], bf, tag="eL")
                nc.sync.dma_start(eL[:], alpha[127:128, :])
                nc.vector.tensor_copy(aL[0:1, 0:NT:2], eL[0:1, 1:NT:2])
                nc.vector.tensor_copy(aL[0:1, 1:NT:2], eL[0:1, 0:NT - 1:2])
                edr = pool.tile([1, NT], bf, tag="edr")
                nc.vector.tensor_copy(edr[0:1, 0:NT:2], alpha[0:1, 1:NT:2])
                nc.vector.tensor_copy(edr[0:1, 1:NT:2], alpha[0:1, 0:NT - 1:2])
                nc.sync.dma_start(aR[127:128, :], edr[:])
                nc.vector.tensor_tensor(aL[:], alpha[:, :], aL[:], op.max)
                PW = aL
                nc.vector.tensor_tensor(PW[:], PW[:], aR[:], op.max)
                z2 = pool.tile([128, NT], bf, tag="z2")
                nc.vector.tensor_tensor(z2[:, 0:NT - 2], PW[:, 0:NT - 2],
                                        PW[:, 2:NT], op.max)
                nc.vector.tensor_tensor(outM[:, 2:NT - 2], z2[:, 0:NT - 4],
                                        PW[:, 4:NT], op.max)
                nc.vector.tensor_tensor(outM[:, 0:2], z2[:, 0:2],
                                        PW[:, NT - 2:NT], op.max)
                nc.vector.tensor_tensor(outM[:, NT - 2:NT], z2[:, NT - 4:NT - 2],
                                        PW[:, 0:2], op.max)
                nc.vector.tensor_scalar(outM[:], outM[:], ALPHA_TH, None, op.is_gt)

            # pipeline: head0, mid0, head1, mid1[0:4], tail0, mid1[4:8],
            # tail1 (tails off the boundary + end critical paths)
            states = {}
            for s in range(SPC):
                states[s] = emit_head(s)
                if s > 0:
                    emit_mid(s, states[s], range(0, 4))
                    emit_tail(s - 1, states.pop(s - 1))
                    emit_mid(s, states[s], range(4, NSTRIP))
                else:
                    emit_mid(s, states[s], range(NSTRIP))
            emit_tail(SPC - 1, states.pop(SPC - 1))

    nc.compile()
    return nc


def _get_built(b1_nonzero, b2_nonzero):
    global _BUILT
    key = (b1_nonzero, b2_nonzero)
    if _BUILT is None or _BUILT[0] != key:
        _BUILT = (key, _build(b1_nonzero, b2_nonzero))
    return _BUILT[1]


# ------------------------------------------------------------------ kernel
def kernel(x, rand_vals, w1, b1, w2, b2):
    from concourse.bass_utils import run_bass_kernel_spmd

    x = np.asarray(x, np.float32)
    rand_vals = np.asarray(rand_vals, np.float32)
    w1e, w2e, b2e = _prep_weights(w1, b1, w2, b2)
    b1_nonzero = bool(np.any(np.asarray(b1, np.float32) != 0.0))
    b2_nonzero = bool(np.any(b2e != 0.0))

    xbf = _prep_xbf(x)
    xcm = _prep_xcm(x)
    xt = _prep_xt(x)
    rt = _prep_randt(rand_vals)

    nc = _get_built(b1_nonzero, b2_nonzero)

    in_maps = []
    for i in range(NCORES):
        sl = slice(SPC * i, SPC * (i + 1))
        in_maps.append({
            "xbf": np.ascontiguousarray(xbf[sl]),
            "xcm": np.ascontiguousarray(xcm[sl]),
            "xt": np.ascontiguousarray(xt[sl]),
            "rt": np.ascontiguousarray(rt[sl]),
            "w1e": w1e, "w2e": w2e,
            "b2e": b2e.reshape(1, 16),
        })

    res = run_bass_kernel_spmd(nc, in_maps, core_ids=list(range(NCORES)))
    global LAST_RESULTS
    LAST_RESULTS = res
    outs = [res.results[i]["outp"] for i in range(NCORES)]
    out_pm = np.concatenate(outs, axis=0)        # [B, 128, 8192] bf16
    return _unprep_out(out_pm)


# revision 23
# speedup vs baseline: 1.0218x; 1.0218x over previous
"""Trainium2 Bass kernel for nn_CAModel (neural cellular automaton step).

Strategy: pure data-parallel over batch (16 samples -> 8 cores x 2).
v3: baseline's contiguous staging (sobel partials materialized shifted so
the S gathers are big-chunk 2D DMAs; x slab streamed from DRAM xcm), plus:
  - x/out in bf16 (halves HBM + update cost; ~0.3% rms, budget is 2e-2)
  - relu split ScalarE(3/4) / VectorE(1/4), b1 folded into the matmul via
    a const-1 row when b1 != 0 (zero in the graded model)
  - x-update on GpSimd, evac at psum-bank granularity
  - per-strip staging, tails emitted last (no FIFO head-of-line blocks)
Host does layout transforms only (free); HW exec time is what counts.
"""

import numpy as np

# ---------------------------------------------------------------- constants
B, C, H, W = 16, 16, 256, 256
NCORES = 8
SPC = B // NCORES          # samples per core
HWPX = H * W               # 65536 pixels per sample
PITCH = 258                # padded row pitch (wrap col + 256 + wrap col)
NROWH = 34                 # rows -1..32 (halo top/bottom) for x_bf
XBF_F = NROWH * PITCH      # 8772
SOB_F = 32 * PITCH         # 8256 (rows 0..31 padded)
PIX_F = 8192               # 32*256 pixels per strip
NT = HWPX // 128           # 512 pixel-tiles per sample
NSTRIP = 8                 # strips of 32 rows
SUB = 1024                 # psum subchunk (one psh tile)
KROWS = 48                 # mm1 contraction: x, V=u(w+1)-u(w-1), E (+1 if b1)
RELU_PAT = (0, 0, 0, 0, 1)  # per-subchunk relu engine: 0=ScalarE, 1=VectorE
ALPHA_TH = 0.1
FIRE = 0.5

_BUILT = None


# ------------------------------------------------------------- host layouts
def _bf16():
    import ml_dtypes
    return ml_dtypes.bfloat16


def _prep_xbf(x):
    """x: [B, C, H, W] f32 -> [B, 128, XBF_F] bf16 strip layout w/ halo+wrap.

    partition p = hb*16 + c ; free = (r, pc): r = hl+1 for hl in -1..32,
    pc: 0 <-> w=255, 1..256 <-> w=0..255, 257 <-> w=0.   h = hb*32 + hl mod 256
    """
    bf16 = _bf16()
    xb = x.astype(bf16)                                   # [B, C, H, W]
    hidx = (np.arange(-1, 33)[None, :] + 32 * np.arange(8)[:, None]) % 256
    xr = xb[:, :, hidx, :]                                # [B, C, 8, 34, W]
    out = np.empty((B, 8, C, NROWH, PITCH), dtype=bf16)
    out[:, :, :, :, 1:257] = np.transpose(xr, (0, 2, 1, 3, 4))
    out[:, :, :, :, 0] = np.transpose(xr[:, :, :, :, 255], (0, 2, 1, 3))
    out[:, :, :, :, 257] = np.transpose(xr[:, :, :, :, 0], (0, 2, 1, 3))
    return np.ascontiguousarray(out.reshape(B, 128, XBF_F))


def _prep_xcm(x):
    """x [B,C,H,W] f32 -> [B, 8, 16, 8192] bf16: per-strip channel-major."""
    bf16 = _bf16()
    xs = x.reshape(B, C, NSTRIP, PIX_F).transpose(0, 2, 1, 3)
    return np.ascontiguousarray(xs.astype(bf16))


def _prep_xt(x):
    """x: [B, C, H, W] f32 -> pixel-major [B, 128, 8192] bf16.

    xt[b, p, 16*t + c] = x[b, c, pix] with pix = 128*t + p (raster order).
    """
    bf16 = _bf16()
    xf = x.reshape(B, C, HWPX).transpose(0, 2, 1)         # [B, pix, C]
    xf = xf.reshape(B, NT, 128, C).transpose(0, 2, 1, 3)  # [B, p, t, c]
    return np.ascontiguousarray(xf.reshape(B, 128, NT * C).astype(bf16))


def _prep_randt(rv):
    """rand_vals [B, 1, H, W] -> [B, 128, NT] f32, rt[b, p, t] = rv[b, pix]."""
    rf = rv.reshape(B, HWPX).reshape(B, NT, 128).transpose(0, 2, 1)
    return np.ascontiguousarray(rf.astype(np.float32))


def _unprep_out(op):
    """out_pm [B, 128, 8192] bf16 -> [B, C, H, W] f32."""
    o = op.astype(np.float32).reshape(B, 128, NT, C).transpose(0, 2, 1, 3)
    o = o.reshape(B, HWPX, C).transpose(0, 2, 1)
    return np.ascontiguousarray(o.reshape(B, C, H, W))


def _prep_weights(w1, b1, w2, b2):
    bf16 = _bf16()
    w1 = np.asarray(w1, np.float32)
    w2 = np.asarray(w2, np.float32)
    b1 = np.asarray(b1, np.float32)
    # S rows: [x; V; E] with u = x(h-1)+2x+x(h+1), V = u(w+1)-u(w-1),
    # d = x(h+1)-x(h-1), E = d(w-1)+2d(w)+d(w+1).
    # pdx = 0.125*V ; pdy = 0.125*E
    wid, wdx, wdy = w1[0::3], w1[1::3], w1[2::3]
    parts = [wid, 0.125 * wdx, 0.125 * wdy]
    if np.any(b1 != 0.0):
        parts.append(b1.reshape(1, 128))
    w1e = np.concatenate(parts, axis=0)                   # [48 or 49, 128]
    # duplicate into rows 64.. for the second PE row-tile
    w1d = np.zeros((128, 128), np.float32)
    kr = w1e.shape[0]
    w1d[0:kr] = w1e
    w1d[64:64 + kr] = w1e
    return (np.ascontiguousarray(w1d.astype(bf16)),
            np.ascontiguousarray(w2.astype(bf16)),
            np.asarray(b2, np.float32).reshape(16))


# ------------------------------------------------------------- build module
def _build(b1_nonzero, b2_nonzero):
    import concourse.bass as bass
    import concourse.bacc as bacc
    import concourse.mybir as mybir
    import concourse.tile as tile

    dt = mybir.dt
    op = mybir.AluOpType
    AF = mybir.ActivationFunctionType

    nc = bacc.Bacc("TRN2", target_bir_lowering=False, debug=False)

    kr = KROWS + (1 if b1_nonzero else 0)
    xbf_d = nc.dram_tensor("xbf", (SPC, 128, XBF_F), dt.bfloat16, kind="ExternalInput")
    xcm_d = nc.dram_tensor("xcm", (SPC, NSTRIP, 16, PIX_F), dt.bfloat16, kind="ExternalInput")
    xt_d = nc.dram_tensor("xt", (SPC, 128, PIX_F), dt.bfloat16, kind="ExternalInput")
    rt_d = nc.dram_tensor("rt", (SPC, 128, NT), dt.float32, kind="ExternalInput")
    w1_d = nc.dram_tensor("w1e", (128, 128), dt.bfloat16, kind="ExternalInput")
    w2_d = nc.dram_tensor("w2e", (128, 16), dt.bfloat16, kind="ExternalInput")
    b2_d = nc.dram_tensor("b2e", (1, 16), dt.float32, kind="ExternalInput")
    out_d = nc.dram_tensor("outp", (SPC, 128, PIX_F), dt.bfloat16, kind="ExternalOutput")

    with tile.TileContext(nc) as tc:
        with (
            tc.tile_pool(name="wpool", bufs=1) as wpool,
            tc.tile_pool(name="xbf", bufs=1) as p_xbf,
            tc.tile_pool(name="pads", bufs=1) as p_pads,
            tc.tile_pool(name="sob", bufs=1) as p_sob,
            tc.tile_pool(name="stage", bufs=4) as p_stage,
            tc.tile_pool(name="hsb", bufs=3) as p_hsb,
            tc.tile_pool(name="xt", bufs=2) as p_xt,
            tc.tile_pool(name="dxm", bufs=2) as p_dxm,
            tc.tile_pool(name="small", bufs=2) as p_small,
            tc.tile_pool(name="small1", bufs=1) as p_small1,
            tc.tile_pool(name="pscr", bufs=1) as p_pscr,
            tc.tile_pool(name="psh", bufs=2, space=bass.MemorySpace.PSUM) as p_psh,
            tc.tile_pool(name="psdx", bufs=2, space=bass.MemorySpace.PSUM) as p_psdx,
        ):
            w1_sb = wpool.tile([128, 128], dt.bfloat16, tag="w1")
            nc.sync.dma_start(w1_sb[:], w1_d.ap())
            w2_sb = wpool.tile([128, 16], dt.bfloat16, tag="w2")
            nc.sync.dma_start(w2_sb[:], w2_d.ap())
            ones_sb = None
            if b1_nonzero:
                ones_sb = wpool.tile([1, PIX_F // 2], dt.bfloat16, tag="ones")
                nc.vector.memset(ones_sb[:], 1.0)
            # gpsimd ucode warmup (first TT call pays ucode load)
            gwarm = wpool.tile([128, 2], dt.bfloat16, tag="gwarm")
            nc.vector.memset(gwarm[:], 0.0)
            nc.gpsimd.tensor_tensor(gwarm[:, 0:1], gwarm[:, 0:1], gwarm[:, 1:2], op.mult)
            nc.gpsimd.tensor_tensor(gwarm[:, 0:1], gwarm[:, 0:1], gwarm[:, 1:2], op.add)
            b2_sb = None
            if b2_nonzero:
                b2_sb = wpool.tile([128, 16], dt.float32, tag="b2")
                nc.sync.dma_start(b2_sb[:], b2_d.ap().broadcast_to([128, 16]))

            def emit_head(s):
                """Loads + sobel partials: UP1/UM1 (shifted u) and E (pdy),
                materialized CONTIGUOUS so staging gathers are cheap DMAs."""
                st = {}
                rt = p_pscr.tile([128, NT], dt.float32, tag="rt")
                nc.sync.dma_start(rt[:], rt_d.ap()[s])
                xbf = p_xbf.tile([128, XBF_F], dt.bfloat16, tag="xbf")
                # load halo rows 0..18 first so sobel half 0 starts early
                nc.scalar.dma_start(xbf[:, 0:18 * PITCH],
                                    xbf_d.ap()[s][:, 0:18 * PITCH])
                nc.scalar.dma_start(xbf[:, 18 * PITCH:],
                                    xbf_d.ap()[s][:, 18 * PITCH:])
                xt = p_xt.tile([128, PIX_F], dt.bfloat16, tag="xt")
                nc.scalar.dma_start(xt[:], xt_d.ap()[s])
                xbf3 = xbf.rearrange("p (r q) -> p r q", q=PITCH)  # [128,34,258]

                V = p_sob.tile([128, PIX_F], dt.bfloat16, tag="V")
                E = p_sob.tile([128, PIX_F], dt.bfloat16, tag="E")
                V3 = V.rearrange("p (r w) -> p r w", w=W)
                E3 = E.rearrange("p (r w) -> p r w", w=W)
                Apad = p_pads.tile([128, SOB_F], dt.bfloat16, tag="A")
                A3 = Apad.rearrange("p (r q) -> p r q", q=PITCH)
                X2 = p_pads.tile([128, SOB_F], dt.bfloat16, tag="X2")
                X23 = X2.rearrange("p (r q) -> p r q", q=PITCH)
                Dpad = p_pads.tile([128, SOB_F], dt.bfloat16, tag="X2")
                D3 = Dpad.rearrange("p (r q) -> p r q", q=PITCH)
                D2 = p_pads.tile([128, SOB_F], dt.bfloat16, tag="A")
                D23 = D2.rearrange("p (r q) -> p r q", q=PITCH)
                for hh in range(2):
                    rr = slice(16 * hh, 16 * hh + 16)
                    x_up = xbf3[:, 16 * hh:16 * hh + 16, :]
                    x_mid = xbf3[:, 16 * hh + 1:16 * hh + 17, :]
                    x_dn = xbf3[:, 16 * hh + 2:16 * hh + 18, :]
                    # d path first (E is the last slab consumers wait on):
                    # D_pad = x_dn - x_up, E = D(w-1)+D(w+1), D2 = 2*D,
                    # E += D2(w)
                    nc.vector.tensor_tensor(D3[:, rr, :], x_dn, x_up, op.subtract)
                    nc.vector.tensor_tensor(E3[:, rr, :], D3[:, rr, 0:256],
                                            D3[:, rr, 2:258], op.add)
                    nc.vector.tensor_scalar(D23[:, rr, :], D3[:, rr, :], 2.0,
                                            None, op.mult)
                    nc.vector.tensor_tensor(E3[:, rr, :], D23[:, rr, 1:257],
                                            E3[:, rr, :], op.add)
                    # u path: U_pad = x_up + x_dn + 2*x_mid (in A's slot),
                    # then V = U_pad(w+1) - U_pad(w-1) contiguous
                    nc.vector.tensor_scalar(X23[:, rr, :], x_mid, 2.0, None, op.mult)
                    nc.vector.tensor_tensor(A3[:, rr, :], x_up, x_dn, op.add)
                    nc.vector.tensor_tensor(A3[:, rr, :], A3[:, rr, :],
                                            X23[:, rr, :], op.add)
                    nc.vector.tensor_tensor(V3[:, rr, :], A3[:, rr, 2:258],
                                            A3[:, rr, 0:256], op.subtract)

                xt3 = xt.rearrange("p (t c) -> p t c", c=C)
                st.update(xt=xt, xt3=xt3, V=V, E=E, rt=rt)
                return st

            def emit_mid(s, st, strips):
                """Per-strip staging (A-block rows 0..kr: even 512-px chunks,
                B-block rows 64..64+kr: odd chunks), row-tiled mm1 pairs,
                relu, mm2, psdx evac at bank (4096 px) granularity."""
                V, E = st["V"], st["E"]
                xt, xt3 = st["xt"], st["xt3"]
                if "alphaN" not in st:
                    alphaN = p_small.tile([128, NT], dt.bfloat16, tag="alN")
                    DA = p_small.tile([128, NT], dt.bfloat16, tag="DA")
                    st["alphaN"] = alphaN
                    st["DA"] = DA
                psdx = None
                state = dict(psh=None, filled=0, cc=0, T=0, psdx=None)
                for hb in strips:
                    pp = slice(16 * hb, 16 * hb + 16)
                    S = p_stage.tile([64 + kr, PIX_F // 2], dt.bfloat16, tag="S")
                    S3 = S.rearrange("p (a e) -> p a e", e=512)   # [.,8,512]
                    xc3 = xcm_d.ap()[s, hb].rearrange("c (a e) -> c a e", e=512)
                    V3s = V[pp, :].rearrange("c (a e) -> c a e", e=512)
                    E3s = E[pp, :].rearrange("c (a e) -> c a e", e=512)
                    # strip 0: stage per half so mm1 starts before the
                    # second sobel half is done
                    halves = ((0, 4), (4, 8)) if hb == 0 else ((0, 8),)
                    for a0, a1 in halves:
                        aa = slice(a0, a1)
                        e0, e1 = 2 * a0, 2 * a1
                        nc.sync.dma_start(S3[0:16, aa], xc3[:, e0:e1:2, :])
                        nc.sync.dma_start(S3[64:80, aa], xc3[:, e0 + 1:e1:2, :])
                        nc.gpsimd.dma_start(S3[16:32, aa], V3s[:, e0:e1:2, :])
                        nc.gpsimd.dma_start(S3[80:96, aa], V3s[:, e0 + 1:e1:2, :])
                        nc.scalar.dma_start(S3[32:48, aa], E3s[:, e0:e1:2, :])
                        nc.scalar.dma_start(S3[96:112, aa], E3s[:, e0 + 1:e1:2, :])
                    if b1_nonzero:
                        nc.sync.dma_start(S[48:49, :], ones_sb[:])
                        nc.sync.dma_start(S[112:113, :], ones_sb[:])

                    for gq in range(16):
                        if state["psh"] is None:
                            state["psh"] = p_psh.tile([128, 1536], dt.float32,
                                                      tag="psh", name="psh")
                            state["filled"] = 0
                        blk, col = gq % 2, 512 * (gq // 2)
                        base = 64 * blk
                        nc.tensor.matmul(
                            state["psh"][:, 512 * state["filled"]:
                                         512 * state["filled"] + 512],
                            w1_sb[base:base + kr, :],
                            S[base:base + kr, col:col + 512])
                        state["filled"] += 1
                        last = hb == strips[-1] and gq == 15
                        if state["filled"] == 3 or last:
                            _flush(s, st, state, last)

            def _flush(s, st, state, last):
                """Relu the filled psh chunk, run mm2 tiles, evac full psdx
                banks."""
                n = 512 * state["filled"]
                psh = state["psh"]
                hsb = p_hsb.tile([128, 1536], dt.bfloat16, tag="hsb")
                cc = state["cc"]
                if cc < 11 or RELU_PAT[cc % len(RELU_PAT)] == 0:
                    nc.scalar.activation(hsb[:, 0:n], psh[:, 0:n], AF.Relu)
                else:
                    nc.vector.tensor_scalar(hsb[:, 0:n], psh[:, 0:n],
                                            0.0, None, op.max)
                state["cc"] = cc + 1
                for t_loc in range(n // 128):
                    T = state["T"]
                    if T % 32 == 0:
                        state["psdx"] = p_psdx.tile([128, 512], dt.float32,
                                                    tag="psdx", name="psdx")
                    tt = T % 32
                    nc.tensor.matmul(
                        state["psdx"][:, 16 * tt:16 * tt + 16],
                        hsb[:, 128 * t_loc:128 * (t_loc + 1)],
                        w2_sb[:])
                    state["T"] = T + 1
                    if T % 32 == 31:
                        bk = T // 32
                        DXM = p_dxm.tile([128, 512], dt.bfloat16, tag="DXM")
                        _evac_bank(nc, state["psdx"], st["rt"], st["xt"],
                                   st["xt3"], DXM, st["alphaN"], st["DA"],
                                   bk, b2_sb, op, dt)
                state["psh"] = None

            def emit_tail(s, st):
                """Post-life pool, life mask, final multiply, store."""
                xt = st["xt"]
                alphaP = p_small1.tile([128, NT], dt.bfloat16, tag="alP")
                nc.vector.tensor_tensor(alphaP[:], st["alphaN"][:], st["DA"][:],
                                        op.subtract)
                preM = p_small1.tile([128, NT], dt.bfloat16, tag="preM")
                _pool_and_thresh(nc, p_pscr, alphaP, preM, op, dt)
                postM = p_small1.tile([128, NT], dt.bfloat16, tag="postM")
                _pool_and_thresh(nc, p_pscr, st["alphaN"], postM, op, dt)
                life = p_small1.tile([128, NT], dt.bfloat16, tag="life")
                eng = nc.vector if s == SPC - 1 else nc.gpsimd
                eng.tensor_tensor(life[:], preM[:], postM[:], op.mult)
                eng.tensor_tensor(
                    xt.rearrange("p (t c) -> p t c", c=C),
                    xt.rearrange("p (t c) -> p t c", c=C),
                    life[:].broadcast_to([128, NT, C]), op.mult)
                nc.gpsimd.dma_start(out_d.ap()[s], xt[:])

            def _evac_bank(nc, psdx, rt, xt, xt3, DXM, alphaN, DA, bk,
                           b2_sb, op, dt):
                """One filled psdx bank (4096 px = 32 tiles): update mask for
                this bank, masked dx -> DXM (bf16), alpha delta -> DA chunk,
                alphaN chunk, x += dx*um in place (bf16)."""
                ps3 = psdx.rearrange("p (t c) -> p t c", c=C)     # [128,32,16]
                sl32 = slice(32 * bk, 32 * bk + 32)
                umk = p_small1.tile([128, 32], dt.bfloat16, tag="umk")
                nc.vector.tensor_scalar(umk[:], rt[:, sl32], FIRE, None, op.is_lt)
                if b2_sb is not None:
                    nc.vector.tensor_tensor(
                        ps3[:], ps3[:],
                        b2_sb[:].rearrange("p c -> p 1 c").broadcast_to([128, 32, C]),
                        op.add)
                dxm3 = DXM.rearrange("p (t c) -> p t c", c=C)
                nc.vector.tensor_tensor(dxm3, ps3[:],
                                        umk[:].broadcast_to([128, 32, C]), op.mult)
                nc.vector.tensor_tensor(DA[:, sl32], ps3[:, :, 3], umk[:], op.mult)
                nc.vector.tensor_tensor(alphaN[:, sl32], DA[:, sl32],
                                        xt3[:, sl32, 3], op.add)
                sl = slice(512 * bk, 512 * (bk + 1))
                nc.gpsimd.tensor_tensor(xt[:, sl], xt[:, sl], DXM[:, :], op.add)

            def _pool_and_thresh(nc, pool, alpha, outM, op, dt):
                """3x3 circular max-pool on pixel-major alpha [128, NT] then
                > ALPHA_TH.  pix = 128*t + p: w-neighbors = partition +-1,
                h-neighbors = free -+2 (parity-interleaved wrap)."""
                bf = dt.bfloat16
                aL = pool.tile([128, NT], bf, tag="aL")
                aR = pool.tile([128, NT], bf, tag="aR")
                nc.sync.dma_start(aL[1:128, :], alpha[0:127, :])
                nc.sync.dma_start(aR[0:127, :], alpha[1:128, :])
                eL = pool.tile([1, NT], bf, tag="eL")
                nc.sync.dma_start(eL[:], alpha[127:128, :])
                nc.vector.tensor_copy(aL[0:1, 0:NT:2], eL[0:1, 1:NT:2])
                nc.vector.tensor_copy(aL[0:1, 1:NT:2], eL[0:1, 0:NT - 1:2])
                edr = pool.tile([1, NT], bf, tag="edr")
                nc.vector.tensor_copy(edr[0:1, 0:NT:2], alpha[0:1, 1:NT:2])
                nc.vector.tensor_copy(edr[0:1, 1:NT:2], alpha[0:1, 0:NT - 1:2])
                nc.sync.dma_start(aR[127:128, :], edr[:])
                nc.vector.tensor_tensor(aL[:], alpha[:, :], aL[:], op.max)
                PW = aL
                nc.vector.tensor_tensor(PW[:], PW[:], aR[:], op.max)
                z2 = pool.tile([128, NT], bf, tag="z2")
                nc.vector.tensor_tensor(z2[:, 0:NT - 2], PW[:, 0:NT - 2],
                                        PW[:, 2:NT], op.max)
                nc.vector.tensor_tensor(outM[:, 2:NT - 2], z2[:, 0:NT - 4],
                                        PW[:, 4:NT], op.max)
                nc.vector.tensor_tensor(outM[:, 0:2], z2[:, 0:2],
                                        PW[:, NT - 2:NT], op.max)
                nc.vector.tensor_tensor(outM[:, NT - 2:NT], z2[:, NT - 4:NT - 2],
                                        PW[:, 0:2], op.max)
                nc.vector.tensor_scalar(outM[:], outM[:], ALPHA_TH, None, op.is_gt)

            # heads+mids first; tails emitted last (avoid FIFO HOL blocking)
            states = {}
            for s in range(SPC):
                states[s] = emit_head(s)
                emit_mid(s, states[s], range(NSTRIP))
            for s in range(SPC):
                emit_tail(s, states.pop(s))

    nc.compile()
    return nc


def _get_built(b1_nonzero, b2_nonzero):
    global _BUILT
    key = (b1_nonzero, b2_nonzero)
    if _BUILT is None or _BUILT[0] != key:
        _BUILT = (key, _build(b1_nonzero, b2_nonzero))
    return _BUILT[1]


# ------------------------------------------------------------------ kernel
def kernel(x, rand_vals, w1, b1, w2, b2):
    from concourse.bass_utils import run_bass_kernel_spmd

    x = np.asarray(x, np.float32)
    rand_vals = np.asarray(rand_vals, np.float32)
    w1e, w2e, b2e = _prep_weights(w1, b1, w2, b2)
    b1_nonzero = bool(np.any(np.asarray(b1, np.float32) != 0.0))
    b2_nonzero = bool(np.any(b2e != 0.0))

    xbf = _prep_xbf(x)
    xcm = _prep_xcm(x)
    xt = _prep_xt(x)
    rt = _prep_randt(rand_vals)

    nc = _get_built(b1_nonzero, b2_nonzero)

    in_maps = []
    for i in range(NCORES):
        sl = slice(SPC * i, SPC * (i + 1))
        in_maps.append({
            "xbf": np.ascontiguousarray(xbf[sl]),
            "xcm": np.ascontiguousarray(xcm[sl]),
            "xt": np.ascontiguousarray(xt[sl]),
            "rt": np.ascontiguousarray(rt[sl]),
            "w1e": w1e, "w2e": w2e,
            "b2e": b2e.reshape(1, 16),
        })

    res = run_bass_kernel_spmd(nc, in_maps, core_ids=list(range(NCORES)))
    global LAST_RESULTS
    LAST_RESULTS = res
    outs = [res.results[i]["outp"] for i in range(NCORES)]
    out_pm = np.concatenate(outs, axis=0)        # [B, 128, 8192] bf16
    return _unprep_out(out_pm)


# revision 25
# speedup vs baseline: 1.0280x; 1.0061x over previous
"""Trainium2 Bass kernel for nn_CAModel (neural cellular automaton step).

Strategy: pure data-parallel over batch (16 samples -> 8 cores x 2).
v3: baseline's contiguous staging (sobel partials materialized shifted so
the S gathers are big-chunk 2D DMAs; x slab streamed from DRAM xcm), plus:
  - x/out in bf16 (halves HBM + update cost; ~0.3% rms, budget is 2e-2)
  - relu split ScalarE(3/4) / VectorE(1/4), b1 folded into the matmul via
    a const-1 row when b1 != 0 (zero in the graded model)
  - x-update on GpSimd, evac at psum-bank granularity
  - per-strip staging, tails emitted last (no FIFO head-of-line blocks)
Host does layout transforms only (free); HW exec time is what counts.
"""

import numpy as np

# ---------------------------------------------------------------- constants
B, C, H, W = 16, 16, 256, 256
NCORES = 8
SPC = B // NCORES          # samples per core
HWPX = H * W               # 65536 pixels per sample
PITCH = 258                # padded row pitch (wrap col + 256 + wrap col)
NROWH = 34                 # rows -1..32 (halo top/bottom) for x_bf
XBF_F = NROWH * PITCH      # 8772
SOB_F = 32 * PITCH         # 8256 (rows 0..31 padded)
PIX_F = 8192               # 32*256 pixels per strip
NT = HWPX // 128           # 512 pixel-tiles per sample
NSTRIP = 8                 # strips of 32 rows
SUB = 1024                 # psum subchunk (one psh tile)
KROWS = 48                 # mm1 contraction: x, V=u(w+1)-u(w-1), E (+1 if b1)
RELU_PAT = (0, 0, 0, 0, 1)  # per-subchunk relu engine: 0=ScalarE, 1=VectorE
ALPHA_TH = 0.1
FIRE = 0.5

_BUILT = None


# ------------------------------------------------------------- host layouts
def _bf16():
    import ml_dtypes
    return ml_dtypes.bfloat16


def _prep_xbf(x):
    """x: [B, C, H, W] f32 -> [B, 128, XBF_F] bf16 strip layout w/ halo+wrap.

    partition p = hb*16 + c ; free = (r, pc): r = hl+1 for hl in -1..32,
    pc: 0 <-> w=255, 1..256 <-> w=0..255, 257 <-> w=0.   h = hb*32 + hl mod 256
    """
    bf16 = _bf16()
    xb = x.astype(bf16)                                   # [B, C, H, W]
    hidx = (np.arange(-1, 33)[None, :] + 32 * np.arange(8)[:, None]) % 256
    xr = xb[:, :, hidx, :]                                # [B, C, 8, 34, W]
    out = np.empty((B, 8, C, NROWH, PITCH), dtype=bf16)
    out[:, :, :, :, 1:257] = np.transpose(xr, (0, 2, 1, 3, 4))
    out[:, :, :, :, 0] = np.transpose(xr[:, :, :, :, 255], (0, 2, 1, 3))
    out[:, :, :, :, 257] = np.transpose(xr[:, :, :, :, 0], (0, 2, 1, 3))
    return np.ascontiguousarray(out.reshape(B, 128, XBF_F))


def _prep_xcm(x):
    """x [B,C,H,W] f32 -> [B, 8, 16, 8192] bf16: per-strip channel-major."""
    bf16 = _bf16()
    xs = x.reshape(B, C, NSTRIP, PIX_F).transpose(0, 2, 1, 3)
    return np.ascontiguousarray(xs.astype(bf16))


def _prep_xt(x):
    """x: [B, C, H, W] f32 -> pixel-major [B, 128, 8192] bf16.

    xt[b, p, 16*t + c] = x[b, c, pix] with pix = 128*t + p (raster order).
    """
    bf16 = _bf16()
    xf = x.reshape(B, C, HWPX).transpose(0, 2, 1)         # [B, pix, C]
    xf = xf.reshape(B, NT, 128, C).transpose(0, 2, 1, 3)  # [B, p, t, c]
    return np.ascontiguousarray(xf.reshape(B, 128, NT * C).astype(bf16))


def _prep_randt(rv):
    """rand_vals [B, 1, H, W] -> [B, 128, NT] f32, rt[b, p, t] = rv[b, pix]."""
    rf = rv.reshape(B, HWPX).reshape(B, NT, 128).transpose(0, 2, 1)
    return np.ascontiguousarray(rf.astype(np.float32))


def _unprep_out(op):
    """out_pm [B, 128, 8192] bf16 -> [B, C, H, W] f32."""
    o = op.astype(np.float32).reshape(B, 128, NT, C).transpose(0, 2, 1, 3)
    o = o.reshape(B, HWPX, C).transpose(0, 2, 1)
    return np.ascontiguousarray(o.reshape(B, C, H, W))


def _prep_weights(w1, b1, w2, b2):
    bf16 = _bf16()
    w1 = np.asarray(w1, np.float32)
    w2 = np.asarray(w2, np.float32)
    b1 = np.asarray(b1, np.float32)
    # S rows: [x; V; E] with u = x(h-1)+2x+x(h+1), V = u(w+1)-u(w-1),
    # d = x(h+1)-x(h-1), E = d(w-1)+2d(w)+d(w+1).
    # pdx = 0.125*V ; pdy = 0.125*E
    wid, wdx, wdy = w1[0::3], w1[1::3], w1[2::3]
    parts = [wid, 0.125 * wdx, 0.125 * wdy]
    if np.any(b1 != 0.0):
        parts.append(b1.reshape(1, 128))
    w1e = np.concatenate(parts, axis=0)                   # [48 or 49, 128]
    # duplicate into rows 64.. for the second PE row-tile
    w1d = np.zeros((128, 128), np.float32)
    kr = w1e.shape[0]
    w1d[0:kr] = w1e
    w1d[64:64 + kr] = w1e
    return (np.ascontiguousarray(w1d.astype(bf16)),
            np.ascontiguousarray(w2.astype(bf16)),
            np.asarray(b2, np.float32).reshape(16))


# ------------------------------------------------------------- build module
def _build(b1_nonzero, b2_nonzero):
    import concourse.bass as bass
    import concourse.bacc as bacc
    import concourse.mybir as mybir
    import concourse.tile as tile

    dt = mybir.dt
    op = mybir.AluOpType
    AF = mybir.ActivationFunctionType

    nc = bacc.Bacc("TRN2", target_bir_lowering=False, debug=False)

    kr = KROWS + (1 if b1_nonzero else 0)
    xbf_d = nc.dram_tensor("xbf", (SPC, 128, XBF_F), dt.bfloat16, kind="ExternalInput")
    xcm_d = nc.dram_tensor("xcm", (SPC, NSTRIP, 16, PIX_F), dt.bfloat16, kind="ExternalInput")
    xt_d = nc.dram_tensor("xt", (SPC, 128, PIX_F), dt.bfloat16, kind="ExternalInput")
    rt_d = nc.dram_tensor("rt", (SPC, 128, NT), dt.float32, kind="ExternalInput")
    w1_d = nc.dram_tensor("w1e", (128, 128), dt.bfloat16, kind="ExternalInput")
    w2_d = nc.dram_tensor("w2e", (128, 16), dt.bfloat16, kind="ExternalInput")
    b2_d = nc.dram_tensor("b2e", (1, 16), dt.float32, kind="ExternalInput")
    out_d = nc.dram_tensor("outp", (SPC, 128, PIX_F), dt.bfloat16, kind="ExternalOutput")

    with tile.TileContext(nc) as tc:
        with (
            tc.tile_pool(name="wpool", bufs=1) as wpool,
            tc.tile_pool(name="xbf", bufs=1) as p_xbf,
            tc.tile_pool(name="pads", bufs=1) as p_pads,
            tc.tile_pool(name="sob", bufs=2) as p_sob,
            tc.tile_pool(name="stage", bufs=4) as p_stage,
            tc.tile_pool(name="hsb", bufs=3) as p_hsb,
            tc.tile_pool(name="xt", bufs=2) as p_xt,
            tc.tile_pool(name="dxm", bufs=2) as p_dxm,
            tc.tile_pool(name="small", bufs=2) as p_small,
            tc.tile_pool(name="small1", bufs=1) as p_small1,
            tc.tile_pool(name="pscr", bufs=1) as p_pscr,
            tc.tile_pool(name="psh", bufs=2, space=bass.MemorySpace.PSUM) as p_psh,
            tc.tile_pool(name="psdx", bufs=2, space=bass.MemorySpace.PSUM) as p_psdx,
        ):
            w1_sb = wpool.tile([128, 128], dt.bfloat16, tag="w1")
            nc.sync.dma_start(w1_sb[:], w1_d.ap())
            w2_sb = wpool.tile([128, 16], dt.bfloat16, tag="w2")
            nc.sync.dma_start(w2_sb[:], w2_d.ap())
            ones_sb = None
            if b1_nonzero:
                ones_sb = wpool.tile([1, PIX_F // 2], dt.bfloat16, tag="ones")
                nc.vector.memset(ones_sb[:], 1.0)
            # gpsimd ucode warmup (first TT call pays ucode load)
            gwarm = wpool.tile([128, 2], dt.bfloat16, tag="gwarm")
            nc.vector.memset(gwarm[:], 0.0)
            nc.gpsimd.tensor_tensor(gwarm[:, 0:1], gwarm[:, 0:1], gwarm[:, 1:2], op.mult)
            nc.gpsimd.tensor_tensor(gwarm[:, 0:1], gwarm[:, 0:1], gwarm[:, 1:2], op.add)
            b2_sb = None
            if b2_nonzero:
                b2_sb = wpool.tile([128, 16], dt.float32, tag="b2")
                nc.sync.dma_start(b2_sb[:], b2_d.ap().broadcast_to([128, 16]))

            def emit_head_loads(s):
                st = {}
                rt = p_pscr.tile([128, NT], dt.float32, tag="rt")
                nc.sync.dma_start(rt[:], rt_d.ap()[s])
                xbf = p_xbf.tile([128, XBF_F], dt.bfloat16, tag="xbf")
                # load halo rows 0..18 first so sobel half 0 starts early
                nc.scalar.dma_start(xbf[:, 0:18 * PITCH],
                                    xbf_d.ap()[s][:, 0:18 * PITCH])
                nc.scalar.dma_start(xbf[:, 18 * PITCH:],
                                    xbf_d.ap()[s][:, 18 * PITCH:])
                xt = p_xt.tile([128, PIX_F], dt.bfloat16, tag="xt")
                nc.scalar.dma_start(xt[:], xt_d.ap()[s])
                V = p_sob.tile([128, PIX_F], dt.bfloat16, tag="V")
                E = p_sob.tile([128, PIX_F], dt.bfloat16, tag="E")
                xt3 = xt.rearrange("p (t c) -> p t c", c=C)
                st.update(xt=xt, xt3=xt3, V=V, E=E, rt=rt, xbf=xbf)
                return st

            def emit_head_sobel(s, st, hh):
                """Sobel partials for rows 16*hh..16*hh+16 of every strip,
                one shared pad (P) via scalar_tensor_tensor."""
                xbf3 = st["xbf"].rearrange("p (r q) -> p r q", q=PITCH)
                V3 = st["V"].rearrange("p (r w) -> p r w", w=W)
                E3 = st["E"].rearrange("p (r w) -> p r w", w=W)
                if "P" not in st:
                    st["P"] = p_pads.tile([128, SOB_F], dt.bfloat16, tag="P", name="P")
                P3 = st["P"].rearrange("p (r q) -> p r q", q=PITCH)
                rr = slice(16 * hh, 16 * hh + 16)
                x_up = xbf3[:, 16 * hh:16 * hh + 16, :]
                x_mid = xbf3[:, 16 * hh + 1:16 * hh + 17, :]
                x_dn = xbf3[:, 16 * hh + 2:16 * hh + 18, :]
                # d path: P = x_dn - x_up; E = P(w-1)+P(w+1); E += 2*P(w)
                nc.vector.tensor_tensor(P3[:, rr, :], x_dn, x_up, op.subtract)
                nc.vector.tensor_tensor(E3[:, rr, :], P3[:, rr, 0:256],
                                        P3[:, rr, 2:258], op.add)
                nc.vector.scalar_tensor_tensor(E3[:, rr, :], P3[:, rr, 1:257],
                                               2.0, E3[:, rr, :],
                                               op.mult, op.add)
                # u path: P = x_up + x_dn; P += 2*x_mid; V = P(w+1)-P(w-1)
                nc.vector.tensor_tensor(P3[:, rr, :], x_up, x_dn, op.add)
                nc.vector.scalar_tensor_tensor(P3[:, rr, :], x_mid, 2.0,
                                               P3[:, rr, :], op.mult, op.add)
                nc.vector.tensor_tensor(V3[:, rr, :], P3[:, rr, 2:258],
                                        P3[:, rr, 0:256], op.subtract)

            def emit_mid(s, st, strips, hooks=None):
                """Per-strip staging (A-block rows 0..kr: even 512-px chunks,
                B-block rows 64..64+kr: odd chunks), row-tiled mm1 pairs,
                relu, mm2, psdx evac at bank (4096 px) granularity."""
                V, E = st["V"], st["E"]
                xt, xt3 = st["xt"], st["xt3"]
                if "alphaN" not in st:
                    alphaN = p_small.tile([128, NT], dt.bfloat16, tag="alN")
                    DA = p_small.tile([128, NT], dt.bfloat16, tag="DA")
                    st["alphaN"] = alphaN
                    st["DA"] = DA
                psdx = None
                state = dict(psh=None, filled=0, cc=0, T=0, psdx=None)
                for hb in strips:
                    pp = slice(16 * hb, 16 * hb + 16)
                    S = p_stage.tile([64 + kr, PIX_F // 2], dt.bfloat16, tag="S")
                    S3 = S.rearrange("p (a e) -> p a e", e=512)   # [.,8,512]
                    xc3 = xcm_d.ap()[s, hb].rearrange("c (a e) -> c a e", e=512)
                    V3s = V[pp, :].rearrange("c (a e) -> c a e", e=512)
                    E3s = E[pp, :].rearrange("c (a e) -> c a e", e=512)
                    # strip 0: stage per half so mm1 starts before the
                    # second sobel half is done
                    halves = ((0, 4), (4, 8)) if hb == 0 else ((0, 8),)
                    for a0, a1 in halves:
                        aa = slice(a0, a1)
                        e0, e1 = 2 * a0, 2 * a1
                        nc.sync.dma_start(S3[0:16, aa], xc3[:, e0:e1:2, :])
                        nc.sync.dma_start(S3[64:80, aa], xc3[:, e0 + 1:e1:2, :])
                        nc.gpsimd.dma_start(S3[16:32, aa], V3s[:, e0:e1:2, :])
                        nc.gpsimd.dma_start(S3[80:96, aa], V3s[:, e0 + 1:e1:2, :])
                        nc.scalar.dma_start(S3[32:48, aa], E3s[:, e0:e1:2, :])
                        nc.scalar.dma_start(S3[96:112, aa], E3s[:, e0 + 1:e1:2, :])
                    if b1_nonzero:
                        nc.sync.dma_start(S[48:49, :], ones_sb[:])
                        nc.sync.dma_start(S[112:113, :], ones_sb[:])

                    for gq in range(16):
                        if state["psh"] is None:
                            state["psh"] = p_psh.tile([128, 1536], dt.float32,
                                                      tag="psh", name="psh")
                            state["filled"] = 0
                        blk, col = gq % 2, 512 * (gq // 2)
                        base = 64 * blk
                        nc.tensor.matmul(
                            state["psh"][:, 512 * state["filled"]:
                                         512 * state["filled"] + 512],
                            w1_sb[base:base + kr, :],
                            S[base:base + kr, col:col + 512])
                        state["filled"] += 1
                        last = hb == strips[-1] and gq == 15
                        if state["filled"] == 3 or last:
                            _flush(s, st, state, last)
                    if hooks and hb in hooks:
                        hooks[hb]()

            def _flush(s, st, state, last):
                """Relu the filled psh chunk, run mm2 tiles, evac full psdx
                banks."""
                n = 512 * state["filled"]
                psh = state["psh"]
                hsb = p_hsb.tile([128, 1536], dt.bfloat16, tag="hsb")
                cc = state["cc"]
                if cc < 11 or RELU_PAT[cc % len(RELU_PAT)] == 0:
                    nc.scalar.activation(hsb[:, 0:n], psh[:, 0:n], AF.Relu)
                else:
                    nc.vector.tensor_scalar(hsb[:, 0:n], psh[:, 0:n],
                                            0.0, None, op.max)
                state["cc"] = cc + 1
                for t_loc in range(n // 128):
                    T = state["T"]
                    if T % 32 == 0:
                        state["psdx"] = p_psdx.tile([128, 512], dt.float32,
                                                    tag="psdx", name="psdx")
                    tt = T % 32
                    nc.tensor.matmul(
                        state["psdx"][:, 16 * tt:16 * tt + 16],
                        hsb[:, 128 * t_loc:128 * (t_loc + 1)],
                        w2_sb[:])
                    state["T"] = T + 1
                    if T % 32 == 31:
                        bk = T // 32
                        DXM = p_dxm.tile([128, 512], dt.bfloat16, tag="DXM")
                        _evac_bank(nc, state["psdx"], st["rt"], st["xt"],
                                   st["xt3"], DXM, st["alphaN"], st["DA"],
                                   bk, b2_sb, op, dt)
                state["psh"] = None

            def emit_tail(s, st):
                """Post-life pool, life mask, final multiply, store."""
                xt = st["xt"]
                alphaP = p_small1.tile([128, NT], dt.bfloat16, tag="alP")
                nc.vector.tensor_tensor(alphaP[:], st["alphaN"][:], st["DA"][:],
                                        op.subtract)
                preM = p_small1.tile([128, NT], dt.bfloat16, tag="preM")
                _pool_and_thresh(nc, p_pscr, alphaP, preM, op, dt)
                postM = p_small1.tile([128, NT], dt.bfloat16, tag="postM")
                _pool_and_thresh(nc, p_pscr, st["alphaN"], postM, op, dt)
                life = p_small1.tile([128, NT], dt.bfloat16, tag="life")
                eng = nc.vector if s == SPC - 1 else nc.gpsimd
                eng.tensor_tensor(life[:], preM[:], postM[:], op.mult)
                eng.tensor_tensor(
                    xt.rearrange("p (t c) -> p t c", c=C),
                    xt.rearrange("p (t c) -> p t c", c=C),
                    life[:].broadcast_to([128, NT, C]), op.mult)
                nc.gpsimd.dma_start(out_d.ap()[s], xt[:])

            def _evac_bank(nc, psdx, rt, xt, xt3, DXM, alphaN, DA, bk,
                           b2_sb, op, dt):
                """One filled psdx bank (4096 px = 32 tiles): update mask for
                this bank, masked dx -> DXM (bf16), alpha delta -> DA chunk,
                alphaN chunk, x += dx*um in place (bf16)."""
                ps3 = psdx.rearrange("p (t c) -> p t c", c=C)     # [128,32,16]
                sl32 = slice(32 * bk, 32 * bk + 32)
                umk = p_small1.tile([128, 32], dt.bfloat16, tag="umk")
                nc.vector.tensor_scalar(umk[:], rt[:, sl32], FIRE, None, op.is_lt)
                if b2_sb is not None:
                    nc.vector.tensor_tensor(
                        ps3[:], ps3[:],
                        b2_sb[:].rearrange("p c -> p 1 c").broadcast_to([128, 32, C]),
                        op.add)
                dxm3 = DXM.rearrange("p (t c) -> p t c", c=C)
                nc.vector.tensor_tensor(dxm3, ps3[:],
                                        umk[:].broadcast_to([128, 32, C]), op.mult)
                nc.vector.tensor_tensor(DA[:, sl32], ps3[:, :, 3], umk[:], op.mult)
                nc.vector.tensor_tensor(alphaN[:, sl32], DA[:, sl32],
                                        xt3[:, sl32, 3], op.add)
                sl = slice(512 * bk, 512 * (bk + 1))
                nc.gpsimd.tensor_tensor(xt[:, sl], xt[:, sl], DXM[:, :], op.add)

            def _pool_and_thresh(nc, pool, alpha, outM, op, dt):
                """3x3 circular max-pool on pixel-major alpha [128, NT] then
                > ALPHA_TH.  pix = 128*t + p: w-neighbors = partition +-1,
                h-neighbors = free -+2 (parity-interleaved wrap)."""
                bf = dt.bfloat16
                aL = pool.tile([128, NT], bf, tag="aL")
                aR = pool.tile([128, NT], bf, tag="aR")
                nc.sync.dma_start(aL[1:128, :], alpha[0:127, :])
                nc.sync.dma_start(aR[0:127, :], alpha[1:128, :])
                eL = pool.tile([1, NT], bf, tag="eL")
                nc.sync.dma_start(eL[:], alpha[127:128, :])
                nc.vector.tensor_copy(aL[0:1, 0:NT:2], eL[0:1, 1:NT:2])
                nc.vector.tensor_copy(aL[0:1, 1:NT:2], eL[0:1, 0:NT - 1:2])
                edr = pool.tile([1, NT], bf, tag="edr")
                nc.vector.tensor_copy(edr[0:1, 0:NT:2], alpha[0:1, 1:NT:2])
                nc.vector.tensor_copy(edr[0:1, 1:NT:2], alpha[0:1, 0:NT - 1:2])
                nc.sync.dma_start(aR[127:128, :], edr[:])
                nc.vector.tensor_tensor(aL[:], alpha[:, :], aL[:], op.max)
                PW = aL
                nc.vector.tensor_tensor(PW[:], PW[:], aR[:], op.max)
                z2 = pool.tile([128, NT], bf, tag="z2")
                nc.vector.tensor_tensor(z2[:, 0:NT - 2], PW[:, 0:NT - 2],
                                        PW[:, 2:NT], op.max)
                nc.vector.tensor_tensor(outM[:, 2:NT - 2], z2[:, 0:NT - 4],
                                        PW[:, 4:NT], op.max)
                nc.vector.tensor_tensor(outM[:, 0:2], z2[:, 0:2],
                                        PW[:, NT - 2:NT], op.max)
                nc.vector.tensor_tensor(outM[:, NT - 2:NT], z2[:, NT - 4:NT - 2],
                                        PW[:, 0:2], op.max)
                nc.vector.tensor_scalar(outM[:], outM[:], ALPHA_TH, None, op.is_gt)

            # next sample's loads+sobel are interleaved into this sample's
            # mid emission (strips 1/3/5) so DVE absorbs them in idle time;
            # tails emitted last (avoid FIFO HOL blocking)
            states = {}
            for s in range(SPC):
                if s not in states:
                    states[s] = emit_head_loads(s)
                    emit_head_sobel(s, states[s], 0)
                    emit_head_sobel(s, states[s], 1)
                hooks = {}
                if s + 1 < SPC:
                    def _mk(sn):
                        def _loads():
                            states[sn] = emit_head_loads(sn)
                        def _sob0():
                            emit_head_sobel(sn, states[sn], 0)
                        def _sob1():
                            emit_head_sobel(sn, states[sn], 1)
                        return _loads, _sob0, _sob1
                    h1, h3, h5 = _mk(s + 1)
                    hooks = {1: h1, 3: h3, 5: h5}
                emit_mid(s, states[s], range(NSTRIP), hooks)
            for s in range(SPC):
                emit_tail(s, states.pop(s))

    nc.compile()
    return nc


def _get_built(b1_nonzero, b2_nonzero):
    global _BUILT
    key = (b1_nonzero, b2_nonzero)
    if _BUILT is None or _BUILT[0] != key:
        _BUILT = (key, _build(b1_nonzero, b2_nonzero))
    return _BUILT[1]


# ------------------------------------------------------------------ kernel
def kernel(x, rand_vals, w1, b1, w2, b2):
    from concourse.bass_utils import run_bass_kernel_spmd

    x = np.asarray(x, np.float32)
    rand_vals = np.asarray(rand_vals, np.float32)
    w1e, w2e, b2e = _prep_weights(w1, b1, w2, b2)
    b1_nonzero = bool(np.any(np.asarray(b1, np.float32) != 0.0))
    b2_nonzero = bool(np.any(b2e != 0.0))

    xbf = _prep_xbf(x)
    xcm = _prep_xcm(x)
    xt = _prep_xt(x)
    rt = _prep_randt(rand_vals)

    nc = _get_built(b1_nonzero, b2_nonzero)

    in_maps = []
    for i in range(NCORES):
        sl = slice(SPC * i, SPC * (i + 1))
        in_maps.append({
            "xbf": np.ascontiguousarray(xbf[sl]),
            "xcm": np.ascontiguousarray(xcm[sl]),
            "xt": np.ascontiguousarray(xt[sl]),
            "rt": np.ascontiguousarray(rt[sl]),
            "w1e": w1e, "w2e": w2e,
            "b2e": b2e.reshape(1, 16),
        })

    res = run_bass_kernel_spmd(nc, in_maps, core_ids=list(range(NCORES)))
    global LAST_RESULTS
    LAST_RESULTS = res
    outs = [res.results[i]["outp"] for i in range(NCORES)]
    out_pm = np.concatenate(outs, axis=0)        # [B, 128, 8192] bf16
    return _unprep_out(out_pm)
